# revision 29
# baseline (speedup 1.0000x reference)
"""CIN (Compressed Interaction Network) kernel for Trainium2, 8-core data parallel.

v3: z-stationary matmuls + duplicated-column phase transposes + 4-ring DMA.

Reference computation (per batch element b, position d):
  hidden = x                                  # (39 fields)
  layer i: z[(m,n)] = x[m] * hidden[n]        # outer product over fields
           cur[o]   = relu(sum_c z[c] W_i[c,o] + b_i[o])   # 200 outs
           hidden, direct = cur[:100], cur[100:]  (layers 0,1);  direct = cur (layer 2)
  out[b, j] = sum_d concat(directs)[j, d]     # (2048, 400)

Strategy: batch sharded across 8 cores (256 batch each, rows = b*32+d -> 8192).
Matmuls run z-STATIONARY: lhsT = z block [K<=125 chans, M=128 rows], rhs =
W [K, 200 outs] streams N=200 cycles.  ReLU writes the h-half into rlh with
its first 100 columns DUPLICATED (rlh[:, j] = h[:, j%100], 203 cols), so the
phase-replicated channel-major tensor ph[p, q, :] = h[(25q+p)%100, :] is a
plain PE matmul transpose of rlh[:, 25q:25q+128] against the identity - no
transpose-mode ops, no selection matmuls.  VectorE (phases 0-2) and GpSimdE
(phase 3) multiply ph against the broadcast-gathered X to form the next
layer's z.  Direct halves reduce over d via tiny [128,16] selection matmuls
into a per-row-tile [16, 400] PSUM accumulator DMA'd straight out.

DMA uses four rings: xr gathers split across gpsimd/scalar/sync, z0 + weights
on vector, ordered by first use; warm-up matmul bursts on a memset tile keep
the PE HAM clock-gate open across the prologue.
"""

import sys

sys.path.insert(0, '/opt/trn_rl_repo')

import numpy as np
import ml_dtypes

import concourse.bacc as bacc
import concourse.mybir as mybir
import concourse.tile as tile
from concourse import bass_utils

BF16 = ml_dtypes.bfloat16

NCORES = 8
B = 2048
BC = B // NCORES          # 256 batch per core
D = 32
ROWS = BC * D             # 8192
F0 = 39
FK = 100
O = 200
RT = 512                  # rows per tile
NRT = ROWS // RT          # 16
BPT = RT // D             # batches per row tile = 16
RB = 128                  # rows per matmul block (stationary M)
NRB = RT // RB            # 4
P0 = 128                  # layer-0 z partitions
KB0 = 7                   # layer-0 K blocks: symmetric z0 -> 780 channels (m<=n),
                          # folded W0'[(m,n)] = W0[m,n] + W0[n,m]; padded to 896
BL = 125                  # layers-1/2: packed K-block size (4-phase H windows)
NBL = 32                  # blocks
NPH = 4                   # H phase offsets {0, 25, 50, 75}
BPP = NBL // NPH          # blocks per phase = 8
PERM = [q + NPH * i for q in range(NPH) for i in range(BPP)]  # phase-major block order
RLW = 2 * FK + 3          # rlh cols: h duplicated so [25q:25q+128] covers phase q

# warm-up matmul burst sizes (N=64 each; sized to PE-cover prologue DMA waits
# without delaying the first real matmuls when DMAs land early)
WARM_A = 72
WARM_B = 560

# broadcast-gather pieces for xr[p, kq] = x[min((125q+p)//100 + 5i, 38), :]
# (q, p0, p1, i0, i1, m_start);  m_start = row for i0 (rows advance by 5)
GATHERS = [
    (0, 0, 100, 0, 8, 0),
    (0, 100, 125, 0, 8, 1),
    (1, 0, 75, 0, 8, 1),
    (1, 75, 125, 0, 8, 2),
    (2, 0, 50, 0, 8, 2),
    (2, 50, 125, 0, 8, 3),
    (3, 0, 25, 0, 8, 3),
    (3, 25, 125, 0, 7, 4),
    (3, 25, 125, 7, 8, 38),   # clamped tail: every p reads row 38
]
# steady-state ring per gather (bytes: gpsimd 1.6MB, scalar 1.6; sync gets
# 0.8MB of gathers + z0 + out = 1.75MB)
XR_ENG = ['gpsimd', 'sync', 'scalar', 'gpsimd', 'gpsimd', 'sync',
          'scalar', 'scalar', 'scalar']
# prologue ring per gather for xr(0), balanced against blob/w0/w1
XR_ENG0 = ['sync', 'sync', 'scalar', 'gpsimd', 'gpsimd', 'sync',
           'gpsimd', 'scalar', 'scalar']
DZ0 = 4               # rowtiles whose z0 is built on-device (prologue ramp)

_cached = {}


def _emit(tc, outs, ins, with_bias=True):
    nc = tc.nc
    z0_d = ins['z0']
    xsrc_d = ins['xsrc']
    xcm_d = ins['xcm']
    w0_d = ins['w0']
    w1_d = ins['w1']
    w2_d = ins['w2']
    blob_d = ins['blob']
    out_d = outs['out']

    bf = mybir.dt.bfloat16
    f32 = mybir.dt.float32
    mult = mybir.AluOpType.mult
    relu = mybir.ActivationFunctionType.Relu

    import contextlib
    ctx = contextlib.ExitStack()
    with ctx:
        const = ctx.enter_context(tc.tile_pool(name="const", bufs=1))
        z0p = ctx.enter_context(tc.tile_pool(name="z0", bufs=2))
        xrp = ctx.enter_context(tc.tile_pool(name="xr", bufs=2))
        zp = ctx.enter_context(tc.tile_pool(name="z", bufs=4))
        php = ctx.enter_context(tc.tile_pool(name="ph", bufs=2))
        rlhp = ctx.enter_context(tc.tile_pool(name="rlh", bufs=4))
        rldp = ctx.enter_context(tc.tile_pool(name="rld", bufs=4))
        outsb = ctx.enter_context(tc.tile_pool(name="outsb", bufs=2))
        mainps = ctx.enter_context(tc.tile_pool(name="mps", bufs=3, space="PSUM"))
        ppsp = ctx.enter_context(tc.tile_pool(name="pps", bufs=2, space="PSUM"))
        accp = ctx.enter_context(tc.tile_pool(name="acc", bufs=2, space="PSUM"))
        wps = ctx.enter_context(tc.tile_pool(name="wps", bufs=1, space="PSUM"))

        # resident weights / consts (small ones packed in one blob: cols
        # ident 0:128 | srb 128:192 | ones 192:320 | bias 320:920 |
        # Sa 920:1816 | Sb 1816:2712  (z0 on-device selection matrices))
        w0_sb = const.tile([P0, KB0, O], bf, tag="w0")
        w1_sb = const.tile([BL, NBL, O], bf, tag="w1")
        w2_sb = const.tile([BL, NBL, O], bf, tag="w2")
        blob_sb = const.tile([128, 2712], bf, tag="blob")
        wtile = const.tile([128, 128], bf, tag="wtile")
        xcmp = ctx.enter_context(tc.tile_pool(name="xcm", bufs=DZ0))
        xasp = ctx.enter_context(tc.tile_pool(name="xasb", bufs=2))
        id_sb = blob_sb[:, 0:128]
        srb_sb = blob_sb[:, 128:192].rearrange("p (a b) -> p a b", b=BPT)
        ones_sb = blob_sb[0:1, 192:320]
        bias_sb = blob_sb[0:1, 320:920].rearrange("p (l c) -> p l c", c=O)
        sa_sb = blob_sb[0:F0, 920:1816].rearrange("p (a b) -> p a b", b=128)
        sb_sb = blob_sb[0:F0, 1816:2712].rearrange("p (a b) -> p a b", b=128)

        xcms = {}

        def load_xcm(rt):
            xcms[rt] = xcmp.tile([F0, RT], bf, tag="xcm", name=f"xcm_{rt}")
            nc.sync.dma_start(xcms[rt], xcm_d[rt])

        def load_z0(rt, engs=('sync', 'sync')):
            if rt < DZ0:
                return dz0(rt)
            z0 = z0p.tile([P0, KB0, RT], bf, tag="z0", name=f"z0_{rt}")
            getattr(nc, engs[0]).dma_start(z0[:, :4, :], z0_d[rt, :, :4, :])
            getattr(nc, engs[1]).dma_start(z0[:, 4:, :], z0_d[rt, :, 4:, :])
            return z0

        def dz0(rt):
            """Build z0(rt) on-device: two selection matmuls gather x rows
            into the (mi, ni) channel layout, VectorE multiplies.  Used for
            the first DZ0 rowtiles to keep their 0.9MB each off the DMA rings
            during the prologue ramp (the PE is idle there anyway)."""
            z0 = z0p.tile([P0, KB0, RT], bf, tag="z0", name=f"z0d_{rt}")
            for kb in range(KB0):
                xa = ppsp.tile([128, RT], f32, tag="pps", name=f"xa_{rt}_{kb}")
                xb = ppsp.tile([128, RT], f32, tag="pps", name=f"xb_{rt}_{kb}")
                nc.tensor.matmul(xa, sa_sb[:, kb, :], xcms[rt], start=True,
                                 stop=True)
                nc.tensor.matmul(xb, sb_sb[:, kb, :], xcms[rt], start=True,
                                 stop=True)
                xasb = xasp.tile([128, RT], bf, tag="xasb",
                                 name=f"xasb_{rt}_{kb}")
                nc.scalar.copy(xasb, xa)
                nc.vector.tensor_tensor(z0[:, kb, :], xasb, xb, mult)
            return z0

        def load_xr(rt, engmap=XR_ENG):
            xr = xrp.tile([BL, NBL, RT], bf, tag="xr", name=f"xr_{rt}")
            for j, (q, p0, p1, i0, i1, _m) in enumerate(GATHERS):
                getattr(nc, engmap[j]).dma_start(
                    xr[p0:p1, q * BPP + i0:q * BPP + i1, :],
                    xsrc_d[rt, j, :i1 - i0, :][None, :, :].to_broadcast(
                        (p1 - p0, i1 - i0, RT)))
            return xr

        def warm(n, name):
            w = wps.tile([128, 64], f32, tag="wps", name=name)
            for _ in range(n):
                nc.tensor.matmul(w, wtile, wtile[:, 0:64], start=True, stop=True)

        def instance(li, rt, zts, accps, xr, carry_in=None, finish=None,
                     eager=False):
            """One layer instance: z-stationary matmuls + post-processing.

            post_h / dsum for rowblock rb are emitted TWO rowblocks behind the
            mains so their LDWEIGHTS never head-block on a just-issued ReLU;
            the tail cluster (rb2/rb3 post-processing + the h1 z-build) is
            returned as a carry closure that the NEXT instance emits after its
            rb0 mains, where the waits resolve under active streaming.
            """
            w_sb = (w0_sb, w1_sb, w2_sb)[li]
            nkb = KB0 if li == 0 else NBL
            if li < 2:
                ph = php.tile([BL, NPH, RT], bf, tag="ph", name=f"ph{li}_{rt}")
                znx = [zp.tile([BL, BPP, RT], bf, tag=f"z{li + 1}",
                               name=f"z{li + 1}_{rt}_{q}") for q in range(NPH)]
            else:
                ph = znx = None
            rlhs = [None] * NRB
            rlds = [None] * NRB
            dsl = slice(0, FK) if li < 2 else slice(0, O)
            dreg = slice(li * FK, (li + 1) * FK) if li < 2 else slice(O, 2 * O)

            def post_h(rb):
                # ph[p, q, rbsl] = h[(25q+p)%100, rbsl] via plain transposes
                # of the duplicated-column rlh.
                rbsl = slice(rb * RB, (rb + 1) * RB)
                pps = ppsp.tile([128, NPH, RB], f32, tag="pps",
                                name=f"pps{li}_{rt}_{rb}")
                for q in range(NPH):
                    nc.tensor.matmul(pps[:, q, :],
                                     rlhs[rb][:, 25 * q:25 * q + 128], id_sb,
                                     start=True, stop=True)
                nc.scalar.copy(ph[:, :, rbsl], pps[0:BL, :, :])

            def dsum(rb):
                nc.tensor.matmul(accps[:, dreg], srb_sb[:, rb, :],
                                 rlds[rb][:, dsl],
                                 start=(rb == 0), stop=(rb == NRB - 1))

            def zbuild_half(h):
                hsl = slice(h * (RT // 2), (h + 1) * (RT // 2))
                for q in range(NPH):
                    nc.vector.tensor_tensor(
                        znx[q][:, :, hsl], xr[:, q * BPP:(q + 1) * BPP, hsl],
                        ph[:, q, hsl][:, None, :].to_broadcast(
                            (BL, BPP, RT // 2)), mult)

            for rb in range(NRB):
                rbsl = slice(rb * RB, (rb + 1) * RB)
                ps = mainps.tile([RB, O], f32, tag="mps", name=f"ps{li}_{rt}_{rb}")
                for kb in range(nkb):
                    if li == 0:
                        lhs = zts[:, kb, rbsl]
                    else:
                        lhs = zts[kb // BPP][:, kb % BPP, rbsl]
                    nc.tensor.matmul(ps, lhs, w_sb[:, kb, :],
                                     start=(kb == 0),
                                     stop=(kb == nkb - 1 and not with_bias))
                if with_bias:
                    nc.tensor.matmul(ps, ones_sb, bias_sb[:, li, :],
                                     start=False, stop=True)
                if li < 2:
                    rlh = rlhp.tile([RB, RLW], bf, tag="rlh",
                                    name=f"rlh{li}_{rt}_{rb}")
                    rld = rldp.tile([RB, FK], bf, tag="rld",
                                    name=f"rld{li}_{rt}_{rb}")
                    nc.scalar.activation(rlh[:, 0:FK], ps[:, 0:FK], relu)
                    nc.scalar.activation(rlh[:, FK:2 * FK], ps[:, 0:FK], relu)
                    nc.scalar.activation(rlh[:, 2 * FK:RLW], ps[:, 0:RLW - 2 * FK],
                                         relu)
                    nc.scalar.activation(rld, ps[:, FK:O], relu)
                else:
                    rlh = None
                    rld = rldp.tile([RB, O], bf, tag="rld",
                                    name=f"rld{li}_{rt}_{rb}")
                    nc.scalar.activation(rld, ps, relu)
                rlhs[rb] = rlh
                rlds[rb] = rld
                if rb == 0 and carry_in is not None:
                    carry_in()
                lag = 1 if eager else 2
                if rb >= lag:
                    if li < 2:
                        post_h(rb - lag)
                    dsum(rb - lag)
                    if eager and li < 2 and rb == 2:
                        zbuild_half(0)
            if eager:
                # last rowtile: no following instance to hide the tail in
                if li < 2:
                    post_h(NRB - 1)
                    zbuild_half(1)
                dsum(NRB - 1)

                def carry_out():
                    if finish is not None:
                        finish()
                return znx, carry_out
            if li < 2:
                zbuild_half(0)

            def carry_out():
                if li < 2:
                    post_h(NRB - 2)
                    dsum(NRB - 2)
                    post_h(NRB - 1)
                    dsum(NRB - 1)
                    zbuild_half(1)
                else:
                    dsum(NRB - 2)
                    dsum(NRB - 1)
                if finish is not None:
                    finish()
            return znx, carry_out

        # ---- prologue.  DMAs ordered by first use across the three rings
        # (gpsimd / scalar / sync); warm-up bursts keep the PE busy (HAM at
        # full clock) while the first inputs stream in.  z0 for the first
        # DZ0 rowtiles is built on-device to unload the rings.
        nc.gpsimd.memset(wtile, 0.0)
        nc.scalar.dma_start(blob_sb, blob_d)
        nc.scalar.dma_start(w0_sb, w0_d)
        for _rt in range(DZ0):
            load_xcm(_rt)
        nc.gpsimd.dma_start(w1_sb, w1_d)
        warm(WARM_A, "warmA")
        z0t = load_z0(0)
        z0t_next = load_z0(1)
        xr_cur = load_xr(0, engmap=XR_ENG0)
        acc_cur = accp.tile([BPT, 2 * O], f32, tag="acc", name="acc_0")
        z1t, carry = instance(0, 0, z0t, acc_cur, xr_cur)
        warm(WARM_B, "warmB")
        xr_next = load_xr(1)
        nc.gpsimd.dma_start(w2_sb, w2_d)

        for rt in range(NRT):
            last = rt == NRT - 1
            z2t, carry = instance(1, rt, z1t, acc_cur, xr_cur, carry_in=carry,
                                  eager=last)
            if not last:
                acc_next = accp.tile([BPT, 2 * O], f32, tag="acc",
                                     name=f"acc_{rt + 1}")
                z1t_next, carry = instance(0, rt + 1, z0t_next, acc_next,
                                           xr_next, carry_in=carry)
            if rt + 2 < NRT:
                xr_nn = load_xr(rt + 2)
            else:
                xr_nn = None

            def _finish(acc=acc_cur, rt=rt):
                ost = outsb.tile([BPT, 2 * O], f32, tag="ost",
                                 name=f"ost_{rt}")
                nc.scalar.copy(ost, acc)
                nc.sync.dma_start(out_d[rt], ost)
            _, carry = instance(2, rt, z2t, acc_cur, xr_cur, carry_in=carry,
                                finish=_finish, eager=last)
            if rt + 2 < NRT:
                z0t_next = load_z0(rt + 2)
            if not last:
                acc_cur = acc_next
                xr_cur = xr_next
                xr_next = xr_nn
                z1t = z1t_next
        carry()          # final ost copy + out DMA


def _pack_w(W):
    Wp = np.zeros((BL * NBL, O), np.float32)
    Wp[:F0 * FK] = W
    return np.ascontiguousarray(
        Wp.reshape(NBL, BL, O)[PERM].transpose(1, 0, 2)).astype(BF16)


def _prep_weights(W0, W1, W2, b0, b1, b2):
    mi, ni = np.triu_indices(F0)
    W0f = W0.reshape(F0, F0, O)
    W0sym = W0f[mi, ni] + np.where((mi != ni)[:, None], W0f[ni, mi], 0.0)
    W0p = np.zeros((P0 * KB0, O), np.float32)
    W0p[:len(mi)] = W0sym
    w0 = np.ascontiguousarray(
        W0p.reshape(KB0, P0, O).transpose(1, 0, 2)).astype(BF16)
    w1 = _pack_w(W1)
    w2 = _pack_w(W2)
    srb = np.zeros((128, NRB, BPT), np.float32)
    r_ = np.arange(128)
    for rb in range(NRB):
        srb[r_, rb, rb * 4 + r_ // D] = 1.0
    blob = np.zeros((128, 2712), np.float32)
    blob[:, 0:128] = np.eye(128)
    blob[:, 128:192] = srb.reshape(128, -1)
    blob[0, 192:320] = 1.0
    blob[0, 320:920] = np.stack([b0, b1, b2]).reshape(-1)
    c_ = np.arange(len(mi))
    sa = np.zeros((F0, KB0 * 128), np.float32)
    sb = np.zeros((F0, KB0 * 128), np.float32)
    sa[mi, c_] = 1.0
    sb[ni, c_] = 1.0
    blob[0:F0, 920:1816] = sa
    blob[0:F0, 1816:2712] = sb
    return w0, w1, w2, blob.astype(BF16)


def _prep_x_shard(x, c):
    xs = x[c * BC:(c + 1) * BC]                           # (BC, 39, 32)
    xt = xs.transpose(1, 0, 2).reshape(F0, ROWS)          # (39, 8192)
    xtr = xt.reshape(F0, NRT, RT)
    # z0: symmetric outer product, upper-triangle channels (m <= n) only;
    # channel c -> (kb, p) with c = 128*kb + p
    mi, ni = np.triu_indices(F0)
    z0 = np.zeros((KB0 * P0, ROWS), np.float32)
    z0[:len(mi)] = xt[mi] * xt[ni]
    z0 = z0.reshape(KB0, P0, NRT, RT)
    z0 = np.ascontiguousarray(z0.transpose(2, 1, 0, 3)).astype(BF16)
    # gather source slabs: xsrc[rt, j, i-i0, :] = x[min(m_start + 5*(i-i0)... , 38)]
    xsrc = np.zeros((NRT, len(GATHERS), 8, RT), np.float32)
    for j, (q, p0, p1, i0, i1, m0) in enumerate(GATHERS):
        for k in range(i1 - i0):
            xsrc[:, j, k, :] = xtr[min(m0 + 5 * k, F0 - 1)]
    xcm = np.ascontiguousarray(xtr.transpose(1, 0, 2)).astype(BF16)
    return {'z0': z0, 'xsrc': xsrc.astype(BF16), 'xcm': xcm}


def _build(with_bias=True):
    key = f'nc_{with_bias}'
    if key in _cached:
        return _cached[key]
    nc = bacc.Bacc("TRN2", target_bir_lowering=False, debug=False,
                   enable_asserts=False, num_devices=NCORES)
    ins = {
        'z0': nc.dram_tensor("z0", (NRT, P0, KB0, RT), mybir.dt.bfloat16,
                             kind="ExternalInput").ap(),
        'xsrc': nc.dram_tensor("xsrc", (NRT, len(GATHERS), 8, RT),
                               mybir.dt.bfloat16, kind="ExternalInput").ap(),
        'w0': nc.dram_tensor("w0", (P0, KB0, O), mybir.dt.bfloat16,
                             kind="ExternalInput").ap(),
        'w1': nc.dram_tensor("w1", (BL, NBL, O), mybir.dt.bfloat16,
                             kind="ExternalInput").ap(),
        'w2': nc.dram_tensor("w2", (BL, NBL, O), mybir.dt.bfloat16,
                             kind="ExternalInput").ap(),
        'blob': nc.dram_tensor("blob", (128, 2712), mybir.dt.bfloat16,
                               kind="ExternalInput").ap(),
        'xcm': nc.dram_tensor("xcm", (NRT, F0, RT), mybir.dt.bfloat16,
                              kind="ExternalInput").ap(),
    }
    outs = {
        'out': nc.dram_tensor("out", (NRT, BPT, 2 * O), mybir.dt.float32,
                              kind="ExternalOutput").ap(),
    }
    with tile.TileContext(nc, trace_sim=False) as tc:
        _emit(tc, outs, ins, with_bias=with_bias)
    nc.compile()
    _cached[key] = nc
    return nc


def kernel(x, W0, W1, W2, b0, b1, b2):
    bias_zero = not (np.any(b0) or np.any(b1) or np.any(b2))
    nc = _build(with_bias=not bias_zero)
    w0, w1, w2, blob = _prep_weights(
        np.asarray(W0, np.float32), np.asarray(W1, np.float32),
        np.asarray(W2, np.float32), np.asarray(b0, np.float32),
        np.asarray(b1, np.float32), np.asarray(b2, np.float32))
    x = np.asarray(x, np.float32)
    in_maps = []
    for c in range(NCORES):
        in_maps.append({
            **_prep_x_shard(x, c),
            'w0': w0, 'w1': w1, 'w2': w2, 'blob': blob,
        })
    res = bass_utils.run_bass_kernel_spmd(
        nc, in_maps, core_ids=list(range(NCORES)))
    out = np.empty((B, 2 * O), np.float32)
    for c in range(NCORES):
        out[c * BC:(c + 1) * BC, :] = res.results[c]['out'].reshape(BC, 2 * O)
    return out


# revision 30
# speedup vs baseline: 1.2437x; 1.2437x over previous
"""CIN (Compressed Interaction Network) kernel for Trainium2, 8-core data parallel.

v2: z-stationary matmul formulation.

Reference computation (per batch element b, position d):
  hidden = x                                  # (39 fields)
  layer i: z[(m,n)] = x[m] * hidden[n]        # outer product over fields
           cur[o]   = relu(sum_c z[c] W_i[c,o] + b_i[o])   # 200 outs
           hidden, direct = cur[:100], cur[100:]  (layers 0,1);  direct = cur (layer 2)
  out[b, j] = sum_d concat(directs)[j, d]     # (2048, 400)

Strategy: batch sharded across 8 cores (256 batch each, rows = b*32+d -> 8192).
Matmuls run z-STATIONARY: lhsT = z block [K<=125 chans, M=128 rows] (M=128
exactly -> fast-weight-load), rhs = W [K, 200 outs] streams N=200 cycles and
covers BOTH output halves in one pass.  Outputs land [rows, 200] in PSUM;
bias is added by one extra K=1 matmul (ones x bias-row); ScalarE applies
ReLU -> SBUF bf16.  The h-half is transposed back to channel-major by PE
transpose, phase-replicated via 0/1 selection matmuls, and multiplied with
the broadcast-gathered X on VectorE to form the next layer's z.  The direct
halves are reduced over d by tiny [128,16] selection matmuls accumulating
into a per-row-tile [16, 400] PSUM accumulator DMA'd straight out.

X replication (xr[p, kq] = x[m(p,kq)]) exploits the phase-major block
permutation: within phase q the source row advances by exactly 5 per block,
so 9 broadcast DMAs per row tile (reading tiny 8-row slabs) build the 4MB
SBUF operand instead of streaming a 65MB host-precomputed gather from HBM.
Layer-0's z = x (x) x is precomputed on the host (it has no on-device
dependency) and DMA'd.
"""

import sys

sys.path.insert(0, '/opt/trn_rl_repo')

import numpy as np
import ml_dtypes

import concourse.bacc as bacc
import concourse.mybir as mybir
import concourse.tile as tile
from concourse import bass_utils

BF16 = ml_dtypes.bfloat16

NCORES = 8
B = 2048
BC = B // NCORES          # 256 batch per core
D = 32
ROWS = BC * D             # 8192
F0 = 39
FK = 100
O = 200
RT = 512                  # rows per tile
NRT = ROWS // RT          # 16
BPT = RT // D             # batches per row tile = 16
RB = 128                  # rows per matmul block (stationary M)
NRB = RT // RB            # 4
P0 = 128                  # layer-0 z partitions
KB0 = 7                   # layer-0 K blocks: symmetric z0 -> 780 channels (m<=n),
                          # folded W0'[(m,n)] = W0[m,n] + W0[n,m]; padded to 896
BL = 125                  # layers-1/2: packed K-block size (4-phase H windows)
NBL = 32                  # blocks
NPH = 4                   # H phase offsets {0, 25, 50, 75}
BPP = NBL // NPH          # blocks per phase = 8
PERM = [q + NPH * i for q in range(NPH) for i in range(BPP)]  # phase-major block order

# broadcast-gather pieces for xr[p, q*8+i, :] = x[min((125q+p)//100 + 5i, 38), :]
# (q, p0, p1, i0, i1, m_start);  m_start = row for i0 (rows advance by 5)
GATHERS = [
    (0, 0, 100, 0, 8, 0),
    (0, 100, 125, 0, 8, 1),
    (1, 0, 75, 0, 8, 1),
    (1, 75, 125, 0, 8, 2),
    (2, 0, 50, 0, 8, 2),
    (2, 50, 125, 0, 8, 3),
    (3, 0, 25, 0, 8, 3),
    (3, 25, 125, 0, 7, 4),
    (3, 25, 125, 7, 8, 38),   # clamped tail: every p reads row 38
]

_cached = {}


def _emit(tc, outs, ins, with_bias=True):
    nc = tc.nc
    z0_d = ins['z0']
    xsrc_d = ins['xsrc']
    w0_d = ins['w0']
    w1_d = ins['w1']
    w2_d = ins['w2']
    blob_d = ins['blob']
    out_d = outs['out']

    bf = mybir.dt.bfloat16
    f32 = mybir.dt.float32
    mult = mybir.AluOpType.mult
    relu = mybir.ActivationFunctionType.Relu

    import contextlib
    ctx = contextlib.ExitStack()
    with ctx:
        const = ctx.enter_context(tc.tile_pool(name="const", bufs=1))
        z0p = ctx.enter_context(tc.tile_pool(name="z0", bufs=2))
        xrp = ctx.enter_context(tc.tile_pool(name="xr", bufs=2))
        zp = ctx.enter_context(tc.tile_pool(name="z", bufs=4))
        php = ctx.enter_context(tc.tile_pool(name="ph", bufs=1))
        hcmp = ctx.enter_context(tc.tile_pool(name="hcm", bufs=1))
        relup = ctx.enter_context(tc.tile_pool(name="relu", bufs=8))
        outsb = ctx.enter_context(tc.tile_pool(name="outsb", bufs=2))
        mainps = ctx.enter_context(tc.tile_pool(name="mps", bufs=3, space="PSUM"))
        tps = ctx.enter_context(tc.tile_pool(name="tps", bufs=1, space="PSUM"))
        selps = ctx.enter_context(tc.tile_pool(name="selps", bufs=2, space="PSUM"))
        accp = ctx.enter_context(tc.tile_pool(name="acc", bufs=2, space="PSUM"))

        # resident weights / consts (small ones packed in one blob: cols
        # ident 0:128 | srb 128:192 | sel 192:704 | ones 704:832 | bias 832:1432)
        w0_sb = const.tile([P0, KB0, O], bf, tag="w0")
        w1_sb = const.tile([BL, NBL, O], bf, tag="w1")
        w2_sb = const.tile([BL, NBL, O], bf, tag="w2")
        blob_sb = const.tile([128, 1440], bf, tag="blob")
        id_sb = blob_sb[:, 0:128]
        srb_sb = blob_sb[:, 128:192].rearrange("p (a b) -> p a b", b=BPT)
        sel_sb = blob_sb[0:FK, 192:704].rearrange("p (q c) -> p q c", c=128)
        ones_sb = blob_sb[0:1, 704:832]
        bias_sb = blob_sb[0:1, 832:1432].rearrange("p (l c) -> p l c", c=O)

        def load_z0(rt):
            z0 = z0p.tile([P0, KB0, RT], bf, tag="z0", name=f"z0_{rt}")
            nc.sync.dma_start(z0[:, :4, :], z0_d[rt, :, :4, :])
            nc.sync.dma_start(z0[:, 4:, :], z0_d[rt, :, 4:, :])
            return z0

        def load_xr(rt, js=None, xr=None, engs=None):
            """Issue gather DMAs js (default all) for row tile rt into xr."""
            if xr is None:
                xr = xrp.tile([BL, NBL, RT], bf, tag="xr", name=f"xr_{rt}")
            for j in (range(len(GATHERS)) if js is None else js):
                q, p0, p1, i0, i1, _m = GATHERS[j]
                if engs is None:
                    eng = nc.gpsimd if j < 5 else nc.sync
                else:
                    eng = engs[j % len(engs)]
                eng.dma_start(
                    xr[p0:p1, q * BPP + i0:q * BPP + i1, :],
                    xsrc_d[rt, j, :i1 - i0, :][None, :, :].to_broadcast(
                        (p1 - p0, i1 - i0, RT)))
            return xr

        def instance(li, rt, zts, accps, xr, hooks=None):
            """One layer instance: z-stationary matmuls + post-processing.

            Small matmuls (transpose/sel/d-sum of rowblock rb) are emitted one
            rowblock behind the mains so they never stall on a just-issued
            ReLU.  sel/z-build run at half-tile width (256 rows): half h of
            the next layer's z feeds exactly rowblocks 2h and 2h+1 there, so
            z is ready well before it is consumed.
            """
            w_sb = (w0_sb, w1_sb, w2_sb)[li]
            nkb = KB0 if li == 0 else NBL
            if li < 2:
                ph = php.tile([BL, NPH, RT], bf, tag=f"ph{li}", name=f"ph{li}_{rt}")
                hcm = hcmp.tile([FK, RT], bf, tag=f"hcm{li}", name=f"hcm{li}_{rt}")
                tp = tps.tile([128, NRB, 128], bf, tag="tp", name=f"tp{li}_{rt}")
                znx = [zp.tile([BL, BPP, RT], bf, tag=f"z{li + 1}",
                               name=f"z{li + 1}_{rt}_{q}") for q in range(NPH)]
            else:
                ph = hcm = tp = znx = None
            rls = [None] * NRB
            dsl = slice(FK, O) if li < 2 else slice(0, O)
            dreg = slice(li * FK, (li + 1) * FK) if li < 2 else slice(O, 2 * O)

            def post_h(rb):
                rbsl = slice(rb * RB, (rb + 1) * RB)
                nc.tensor.transpose(tp[:, rb, :], rls[rb][:, 0:128], id_sb)
                nc.scalar.copy(hcm[:, rbsl], tp[0:FK, rb, :])

            def dsum(rb):
                nc.tensor.matmul(accps[:, dreg], srb_sb[:, rb, :], rls[rb][:, dsl],
                                 start=(rb == 0), stop=(rb == NRB - 1))

            def sel_half(h):
                hsl = slice(h * (RT // 2), (h + 1) * (RT // 2))
                for q in range(NPH):
                    sps = selps.tile([128, RT // 2], f32, tag="selps",
                                     name=f"sps{li}_{rt}_{h}_{q}")
                    nc.tensor.matmul(sps, sel_sb[:, q, :], hcm[:, hsl],
                                     start=True, stop=True)
                    nc.scalar.copy(ph[:, q, hsl], sps[0:BL, :])
                    nc.vector.tensor_tensor(
                        znx[q][:, :, hsl], xr[:, q * BPP:(q + 1) * BPP, hsl],
                        ph[:, q, hsl][:, None, :].to_broadcast(
                            (BL, BPP, RT // 2)), mult)

            for rb in range(NRB):
                rbsl = slice(rb * RB, (rb + 1) * RB)
                ps = mainps.tile([RB, O], f32, tag="mps", name=f"ps{li}_{rt}_{rb}")
                for kb in range(nkb):
                    if li == 0:
                        lhs = zts[:, kb, rbsl]
                    else:
                        lhs = zts[kb // BPP][:, kb % BPP, rbsl]
                    nc.tensor.matmul(ps, lhs, w_sb[:, kb, :],
                                     start=(kb == 0),
                                     stop=(kb == nkb - 1 and not with_bias))
                if with_bias:
                    nc.tensor.matmul(ps, ones_sb, bias_sb[:, li, :],
                                     start=False, stop=True)
                rl = relup.tile([RB, O], bf, tag="relu", name=f"rl{li}_{rt}_{rb}")
                nc.scalar.activation(rl, ps, relu)
                rls[rb] = rl
                if rb >= 1:
                    if li < 2:
                        post_h(rb - 1)
                    dsum(rb - 1)
                    if li < 2 and rb == 2:
                        sel_half(0)
                if hooks and rb in hooks:
                    hooks[rb]()
            if li < 2:
                post_h(NRB - 1)
                sel_half(1)
            dsum(NRB - 1)
            return znx

        # ---- prologue.  DMAs split 3 ways; PE warms the HAM clock gate with
        # dummy matmuls on the identity tile while the first inputs stream in.
        nc.scalar.dma_start(blob_sb, blob_d)
        nc.sync.dma_start(w0_sb, w0_d)
        z0t = load_z0(0)
        xr_cur = load_xr(0, engs=[nc.gpsimd, nc.sync, nc.scalar])
        nc.gpsimd.dma_start(w1_sb, w1_d)
        nc.scalar.dma_start(w2_sb, w2_d)
        wt = mainps.tile([RB, O], f32, tag="mps", name="warm")
        for i in range(96):
            nc.tensor.matmul(wt[:, 0:64], id_sb, id_sb[:, 0:64],
                             start=True, stop=True)
        acc_cur = accp.tile([BPT, 2 * O], f32, tag="acc", name="acc_0")
        z1t = instance(0, 0, z0t, acc_cur, xr_cur)
        z0t_next = load_z0(1)
        xr_next = load_xr(1)

        for rt in range(NRT):
            z2t = instance(1, rt, z1t, acc_cur, xr_cur)
            if rt + 1 < NRT:
                acc_next = accp.tile([BPT, 2 * O], f32, tag="acc",
                                     name=f"acc_{rt + 1}")
                z1t = instance(0, rt + 1, z0t_next, acc_next, xr_next)
            # prefetch rt+2 from inside the L2 instance, after the consumers
            # of the buffers being overwritten have drained (waits ~0 there,
            # and gpsimd/sync queues carry nothing latency-critical).
            hooks = {}
            if rt + 2 < NRT:
                xr_nn = xrp.tile([BL, NBL, RT], bf, tag="xr",
                                 name=f"xr_{rt + 2}")
                hooks[1] = lambda: load_xr(rt + 2, js=range(0, 5), xr=xr_nn)
                hooks[3] = lambda: load_xr(rt + 2, js=range(5, 9), xr=xr_nn)
            else:
                xr_nn = None
            instance(2, rt, z2t, acc_cur, xr_cur, hooks=hooks)
            ost = outsb.tile([BPT, 2 * O], f32, tag="ost", name=f"ost_{rt}")
            nc.scalar.copy(ost, acc_cur)
            nc.sync.dma_start(out_d[rt], ost)
            if rt + 2 < NRT:
                z0t_next = load_z0(rt + 2)
            if rt + 1 < NRT:
                acc_cur = acc_next
                xr_cur = xr_next
                xr_next = xr_nn


def _pack_w(W):
    Wp = np.zeros((BL * NBL, O), np.float32)
    Wp[:F0 * FK] = W
    return np.ascontiguousarray(
        Wp.reshape(NBL, BL, O)[PERM].transpose(1, 0, 2)).astype(BF16)


def _prep_weights(W0, W1, W2, b0, b1, b2):
    mi, ni = np.triu_indices(F0)
    W0f = W0.reshape(F0, F0, O)
    W0sym = W0f[mi, ni] + np.where((mi != ni)[:, None], W0f[ni, mi], 0.0)
    W0p = np.zeros((P0 * KB0, O), np.float32)
    W0p[:len(mi)] = W0sym
    w0 = np.ascontiguousarray(
        W0p.reshape(KB0, P0, O).transpose(1, 0, 2)).astype(BF16)
    w1 = _pack_w(W1)
    w2 = _pack_w(W2)
    sel = np.zeros((FK, NPH, 128), np.float32)
    q_, p_ = np.meshgrid(np.arange(NPH), np.arange(BL), indexing='ij')
    sel[(25 * q_ + p_) % FK, q_, p_] = 1.0
    srb = np.zeros((128, NRB, BPT), np.float32)
    r_ = np.arange(128)
    for rb in range(NRB):
        srb[r_, rb, rb * 4 + r_ // D] = 1.0
    blob = np.zeros((128, 1440), np.float32)
    blob[:, 0:128] = np.eye(128)
    blob[:, 128:192] = srb.reshape(128, -1)
    blob[:FK, 192:704] = sel.reshape(FK, -1)
    blob[0, 704:832] = 1.0
    blob[0, 832:1432] = np.stack([b0, b1, b2]).reshape(-1)
    return w0, w1, w2, blob.astype(BF16)


def _prep_x_shard(x, c):
    xs = x[c * BC:(c + 1) * BC]                           # (BC, 39, 32)
    xt = xs.transpose(1, 0, 2).reshape(F0, ROWS)          # (39, 8192)
    xtr = xt.reshape(F0, NRT, RT)
    # z0: symmetric outer product, upper-triangle channels (m <= n) only;
    # channel c -> (kb, p) with c = 128*kb + p
    mi, ni = np.triu_indices(F0)
    z0 = np.zeros((KB0 * P0, ROWS), np.float32)
    z0[:len(mi)] = xt[mi] * xt[ni]
    z0 = z0.reshape(KB0, P0, NRT, RT)
    z0 = np.ascontiguousarray(z0.transpose(2, 1, 0, 3)).astype(BF16)
    # gather source slabs: xsrc[rt, j, i-i0, :] = x[min(m_start + 5*(i-i0)... , 38)]
    xsrc = np.zeros((NRT, len(GATHERS), 8, RT), np.float32)
    for j, (q, p0, p1, i0, i1, m0) in enumerate(GATHERS):
        for k in range(i1 - i0):
            xsrc[:, j, k, :] = xtr[min(m0 + 5 * k, F0 - 1)]
    return {'z0': z0, 'xsrc': xsrc.astype(BF16)}


def _build(with_bias=True):
    key = f'nc_{with_bias}'
    if key in _cached:
        return _cached[key]
    nc = bacc.Bacc("TRN2", target_bir_lowering=False, debug=False,
                   enable_asserts=False, num_devices=NCORES)
    ins = {
        'z0': nc.dram_tensor("z0", (NRT, P0, KB0, RT), mybir.dt.bfloat16,
                             kind="ExternalInput").ap(),
        'xsrc': nc.dram_tensor("xsrc", (NRT, len(GATHERS), 8, RT),
                               mybir.dt.bfloat16, kind="ExternalInput").ap(),
        'w0': nc.dram_tensor("w0", (P0, KB0, O), mybir.dt.bfloat16,
                             kind="ExternalInput").ap(),
        'w1': nc.dram_tensor("w1", (BL, NBL, O), mybir.dt.bfloat16,
                             kind="ExternalInput").ap(),
        'w2': nc.dram_tensor("w2", (BL, NBL, O), mybir.dt.bfloat16,
                             kind="ExternalInput").ap(),
        'blob': nc.dram_tensor("blob", (128, 1440), mybir.dt.bfloat16,
                               kind="ExternalInput").ap(),
    }
    outs = {
        'out': nc.dram_tensor("out", (NRT, BPT, 2 * O), mybir.dt.float32,
                              kind="ExternalOutput").ap(),
    }
    with tile.TileContext(nc, trace_sim=False) as tc:
        _emit(tc, outs, ins, with_bias=with_bias)
    nc.compile()
    _cached[key] = nc
    return nc


def kernel(x, W0, W1, W2, b0, b1, b2):
    bias_zero = not (np.any(b0) or np.any(b1) or np.any(b2))
    nc = _build(with_bias=not bias_zero)
    w0, w1, w2, blob = _prep_weights(
        np.asarray(W0, np.float32), np.asarray(W1, np.float32),
        np.asarray(W2, np.float32), np.asarray(b0, np.float32),
        np.asarray(b1, np.float32), np.asarray(b2, np.float32))
    x = np.asarray(x, np.float32)
    in_maps = []
    for c in range(NCORES):
        in_maps.append({
            **_prep_x_shard(x, c),
            'w0': w0, 'w1': w1, 'w2': w2, 'blob': blob,
        })
    res = bass_utils.run_bass_kernel_spmd(
        nc, in_maps, core_ids=list(range(NCORES)))
    out = np.empty((B, 2 * O), np.float32)
    for c in range(NCORES):
        out[c * BC:(c + 1) * BC, :] = res.results[c]['out'].reshape(BC, 2 * O)
    return out

